# revision 23
# baseline (speedup 1.0000x reference)
"""Trainium2 Bass kernel for a segmented tensor-product contraction.

Computation (per batch row z, channel u, segments of width U=128):
  out[z, so, u] += c_p * x0[i0[z], s0_p, u] * prod_k x1[z, sk_p, u]
for 256 paths of degree 1..3 over S=16 segments.

Strategy:
  - Data-parallel over z across 8 NeuronCores (512 rows each).
  - On-chip layout: [u (partitions) x z (free dim)] per segment; every
    elementwise op is a [128, 512] instruction.
  - x0 row gather: host builds one-hot(i0) per core; TensorEngine computes
    x0gT[s] = x0[:, s]^T @ onehot (gather + transpose for free).
  - Factorization (globally optimized per so-group): suffix products
    sg(s0,s) = x0g[s0]*x1[s] and pairs pr(a,b) = x1[a]*x1[b]; each path is
    one tensor_tensor plus a coefficient scale on ScalarE (or a fused
    scalar_tensor_tensor on VectorE for a fraction of paths).
  - Product builds are packed into merged multi-segment instructions
    (sg runs share one instr via a stride-0 broadcast of x0g[s0]; pair
    runs along constant delta read contiguous x1 spans).
  - Output accumulation on TensorEngine: identity-matmul each path term
    into a per-segment PSUM bank (exact f32 adds). 16 output segments ->
    two groups of 8 banks; the so-partition is optimized to minimize
    duplicated product builds.
"""

import os
from collections import defaultdict

import numpy as np

U = 128
S = 16
NELEM = 64
Z = 4096
NCORES = 8
ZS = Z // NCORES  # 512 rows per core

LAST_EXEC_NS = None
LAST_RESULTS = None

F32 = "float32"


def _parse_paths(idxs, coeffs):
    paths = []  # (degree, x1segs_sorted, s0, so, coeff)
    for idx, cf in zip(idxs, coeffs):
        d = idx.shape[1] - 2
        for r, c in zip(idx, cf):
            r = [int(v) for v in r]
            paths.append((d, tuple(sorted(r[:d])), r[d], r[d + 1], float(c)))
    return paths


def _options(p):
    """Candidate (products, form) decompositions for a path.

    Each option: (frozenset of product keys, form)
    form = (in0_ref, in1_ref) with refs ('x1',s) ('x0g',s) ('sg',(s0,s))
    ('pair',(a,b)); d1 form = (('sg',(s0,s)), None).
    Product keys: ('sg',(s0,s)), ('pair',(a,b)).
    """
    d, segs, s0, so, c = p
    if d == 1:
        k = ("sg", (s0, segs[0]))
        return [(frozenset([k]), (k, None))]
    if d == 2:
        a, b = segs
        opts = [
            (frozenset([("sg", (s0, b))]), (("x1", a), ("sg", (s0, b)))),
            (frozenset([("sg", (s0, a))]), (("x1", b), ("sg", (s0, a)))),
            (frozenset([("pair", (a, b))]), (("pair", (a, b)), ("x0g", s0))),
        ]
        return opts
    a, b, cc = segs
    return [
        (
            frozenset([("pair", (a, b)), ("sg", (s0, cc))]),
            (("pair", (a, b)), ("sg", (s0, cc))),
        ),
        (
            frozenset([("pair", (a, cc)), ("sg", (s0, b))]),
            (("pair", (a, cc)), ("sg", (s0, b))),
        ),
        (
            frozenset([("pair", (b, cc)), ("sg", (s0, a))]),
            (("pair", (b, cc)), ("sg", (s0, a))),
        ),
    ]


def _optimize_group(gpaths, n_sweeps=4):
    """Choose per-path decomposition minimizing total unique products."""
    choices = [0] * len(gpaths)
    opts = [_options(p) for p in gpaths]
    for _ in range(n_sweeps):
        counts = defaultdict(int)
        for i, p in enumerate(gpaths):
            for k in opts[i][choices[i]][0]:
                counts[k] += 1
        changed = False
        for i, p in enumerate(gpaths):
            best, best_cost = choices[i], None
            for j, (prods, _) in enumerate(opts[i]):
                # marginal cost: products not used by anyone else
                cost = 0.0
                for k in prods:
                    others = counts[k] - (1 if k in opts[i][choices[i]][0] else 0)
                    cost += 1.0 / (1 + others)
                if best_cost is None or cost < best_cost - 1e-9:
                    best, best_cost = j, cost
            if best != choices[i]:
                # update counts incrementally
                for k in opts[i][choices[i]][0]:
                    counts[k] -= 1
                for k in opts[i][best][0]:
                    counts[k] += 1
                choices[i] = best
                changed = True
        if not changed:
            break
    products = set()
    forms = []
    for i, p in enumerate(gpaths):
        prods, form = opts[i][choices[i]]
        products |= prods
        forms.append(form)
    return products, forms


def _group_cost(paths, sos_a):
    """Estimate total builds for a candidate so-partition."""
    total = 0
    for sos in (sos_a, [s for s in range(S) if s not in sos_a]):
        gp = [p for p in paths if p[3] in sos]
        prods, _ = _optimize_group(gp, n_sweeps=4)
        total += len(prods)
    return total


def _optimize_partition(paths):
    """Two-stage exhaustive search of the 8/8 so-partition (C(16,8)/2 =
    6435 candidates): cheap 1-sweep proxy scan, then exact re-scoring of
    the best candidates."""
    from itertools import combinations

    def cost(sos_a, sweeps):
        total = 0
        for sos in (sos_a, [s for s in range(S) if s not in sos_a]):
            gp = [p for p in paths if p[3] in sos]
            prods, _ = _optimize_group(gp, n_sweeps=sweeps)
            total += len(prods)
        return total

    cands = [c for c in combinations(range(S), 8) if 0 in c]
    scored = sorted(cands, key=lambda c: cost(list(c), 1))[:30]
    best = min(scored, key=lambda c: cost(list(c), 4))
    cur = list(best)
    other = [s for s in range(S) if s not in cur]
    return cur, other


def _plan_merges(products):
    """Pack product builds into merged instructions.

    Returns (slot_of, builds) where slot_of maps product key -> slot index
    and builds is a list of ('sg_run', s0, s_lo, n, slot_lo) or
    ('pair_run', delta, a_lo, n, slot_lo).
    """
    slot_of = {}
    builds = []
    next_slot = 0
    sgs = defaultdict(list)  # s0 -> sorted s list
    prs = defaultdict(list)  # delta -> sorted a list
    for k in products:
        if k[0] == "sg":
            sgs[k[1][0]].append(k[1][1])
        else:
            a, b = k[1]
            prs[b - a].append(a)
    squares = sorted(prs.pop(0, []))
    # pair runs first: they depend only on x1t (no gather chain), so the
    # DVE can start on them while the x0 gather pipeline fills
    for delta in sorted(prs):
        aa = sorted(prs[delta])
        run = [aa[0]]
        for a in aa[1:] + [None]:
            if a is not None and a == run[-1] + 1:
                run.append(a)
            else:
                kind = "pair_run"
                builds.append((kind, delta, run[0], len(run), next_slot))
                for i, ra in enumerate(run):
                    slot_of[("pair", (ra, ra + delta))] = next_slot + i
                next_slot += len(run)
                if a is not None:
                    run = [a]
    for s0 in sorted(sgs):
        ss = sorted(sgs[s0])
        run = [ss[0]]
        for s in ss[1:] + [None]:
            if s is not None and s == run[-1] + 1:
                run.append(s)
            else:
                builds.append(("sg_run", s0, run[0], len(run), next_slot))
                for i, rs in enumerate(run):
                    slot_of[("sg", (s0, rs))] = next_slot + i
                next_slot += len(run)
                if s is not None:
                    run = [s]
    return slot_of, builds, next_slot, squares


def _build_plan(idxs, coeffs):
    """Full schedule. Returns (groups, all_sq).

    Joint factorization over ALL paths; products used by both so-groups
    are built once into a shared slot region and stay resident across
    both PSUM phases. Group-unique products overlay one reuse region.
    """
    paths = _parse_paths(idxs, coeffs)
    products, forms = _optimize_group(paths, n_sweeps=6)
    part_a = list(range(8))
    part_b = list(range(8, 16))

    all_sq = sorted(
        set(k[1][0] for k in products if k[0] == "pair" and k[1][0] == k[1][1])
    )
    sq_keys = set(("pair", (s, s)) for s in all_sq)

    # classify products by which groups use them
    use_a, use_b = set(), set()
    for p, form in zip(paths, forms):
        tgt = use_a if p[3] in part_a else use_b
        for r in form:
            if r and r[0] in ("sg", "pair") and r not in sq_keys:
                tgt.add(r)
    shared = use_a & use_b
    uniq = {0: use_a - shared, 1: use_b - shared}

    slot_shared, builds_shared, ns, _ = _plan_merges(shared)
    slot_a, builds_a, na, _ = _plan_merges(uniq[0])
    slot_b, builds_b, nb, _ = _plan_merges(uniq[1])
    base2 = ns
    n_main = ns + max(na, nb)
    sq_slot = {s: n_main + i for i, s in enumerate(all_sq)}
    n_slots = n_main + len(all_sq)

    def shift(builds, slot, delta):
        bs = [(b[0], b[1], b[2], b[3], b[4] + delta) for b in builds]
        sl = {k: v + delta for k, v in slot.items()}
        return bs, sl

    builds_a, slot_a = shift(builds_a, slot_a, base2)
    builds_b, slot_b = shift(builds_b, slot_b, base2)

    groups = []
    for gi, (sos, gbuilds, gslot) in enumerate(
        (
            (part_a, builds_shared + builds_a, {**slot_shared, **slot_a}),
            (part_b, builds_b, {**slot_shared, **slot_b}),
        )
    ):
        slot_of = dict(gslot)
        for s in all_sq:
            slot_of[("pair", (s, s))] = sq_slot[s]
        gidx = [i for i, p in enumerate(paths) if p[3] in sos]
        order = sorted(
            gidx,
            key=lambda i: (
                paths[i][0] != 1,
                max(
                    (
                        slot_of[r]
                        for r in forms[i]
                        if r and r[0] in ("sg", "pair")
                    ),
                    default=-1,
                ),
            ),
        )
        path_ops = [
            (paths[i][0], forms[i][0], forms[i][1], paths[i][4], paths[i][3])
            for i in order
        ]
        groups.append(
            dict(
                sos=sos,
                builds=gbuilds,
                slot_of=slot_of,
                n_slots=n_slots,
                path_ops=path_ops,
            )
        )
    return groups, all_sq


SLAB = 32  # coefficient-diagonal matrices per DMA slab


def _build_bass(groups, dtype_name, act_frac, warmup, pool_frac=0.0, all_sq=(), gpsimd_every=0):
    import concourse.bacc as bacc
    import concourse.mybir as mybir
    from concourse.tile import TileContext

    dt = mybir.dt.float32 if dtype_name == F32 else mybir.dt.bfloat16
    MULT = mybir.AluOpType.mult

    nc = bacc.Bacc("TRN2", debug=False)

    n_paths_total = sum(len(g["path_ops"]) for g in groups)
    n_slabs = (n_paths_total + SLAB - 1) // SLAB

    x1t_d = nc.dram_tensor("x1t", [S * U, ZS], dt, kind="ExternalInput")
    x0_d = nc.dram_tensor("x0w", [NELEM, S * U], dt, kind="ExternalInput")
    oh_d = nc.dram_tensor("oh", [NELEM, ZS], dt, kind="ExternalInput")
    cd_d = nc.dram_tensor("cdiag", [n_slabs * SLAB * U, U], dt, kind="ExternalInput")
    out_d = nc.dram_tensor("outt", [S * U, ZS], dt, kind="ExternalOutput")
    junk_d = nc.dram_tensor("junk", [U, ZS], mybir.dt.float32)

    max_slots = max(g["n_slots"] for g in groups)
    coeff_order = []  # flat list of coefficients in emission order

    MAXRUN = 4
    MAXSTRIDE = 63  # ISA: 16-bit step_elem field caps r-stride at 32767 elems
    TMP_BUFS = {1: 10, 2: 4, 3: 2, 4: 3}

    with TileContext(nc) as tc:
        with tc.tile_pool(name="persist", bufs=1) as persist, tc.tile_pool(
            name="tmp", bufs=6
        ) as tmp_pool, tc.tile_pool(name="slab", bufs=2) as slab_pool:
            x1t = persist.tile([U, S * ZS], dt, tag="x1t")
            x0g = persist.tile([U, S * ZS], dt, tag="x0g")
            prod = persist.tile([U, max_slots * ZS], dt, tag="prod")
            x0_sb = persist.tile([NELEM, S * U], dt, tag="x0w")
            oh_sb = persist.tile([NELEM, ZS], dt, tag="oh")
            warm_sb = persist.tile([NELEM, ZS], dt, tag="warmsrc")

            def seg(t, s):
                return t[:, s * ZS : (s + 1) * ZS]

            def span(t, lo, n):
                return t[:, lo * ZS : (lo + n) * ZS]

            # x1t DRAM rows are host-permuted into first-use order
            # (seg_order); SBUF position k holds segment seg_order[k]. DMA in
            # four contiguous 4-segment chunks on one queue (cross-queue DMAs
            # into one tile serialize on a WAW dependency).
            seg_order = []

            def _want(s):
                if s not in seg_order:
                    seg_order.append(s)

            for g in groups:
                for b in g["builds"]:
                    kind, key, lo, n, _ = b
                    if kind == "pair_run":
                        for i in range(n):
                            _want(lo + i)
                            _want(lo + i + key)
                    else:
                        for i in range(n):
                            _want(lo + i)
            for s in all_sq:
                _want(s)
            for s in range(S):
                _want(s)
            posmap = {s: s for s in range(S)}  # identity: natural DRAM order
            for s in seg_order[:2]:
                nc.sync.dma_start(
                    out=seg(x1t, s), in_=x1t_d[s * U : (s + 1) * U, :]
                )
            nc.scalar.dma_start(out=oh_sb[:], in_=oh_d[:])
            nc.scalar.dma_start(out=x0_sb[:], in_=x0_d[:])
            for k, s in enumerate(seg_order[2:]):
                eng = nc.sync if k % 2 == 0 else nc.scalar
                eng.dma_start(
                    out=seg(x1t, s), in_=x1t_d[s * U : (s + 1) * U, :]
                )

            # global square products on ScalarE (before any ACT Copy use,
            # emitted as consecutive runs to avoid table-set thrashing)
            if all_sq:
                max_g = groups[0]["n_slots"] - len(all_sq)
                for ri, s in enumerate(all_sq):
                    nc.scalar.activation(
                        span(prod, max_g + ri, 1),
                        seg(x1t, posmap[s]),
                        mybir.ActivationFunctionType.Square,
                    )

            nc.gpsimd.memset(warm_sb[:], 0.0)
            # PE warmup burst (reads a memset tile: no DMA dependency, keeps
            # the HAM clock-gate warm before the real stream starts)
            # + the 16 x0 gather matmuls
            with tc.tile_pool(name="gpsum", bufs=4, space="PSUM") as gpsum:
                if warmup > 0:
                    wt = gpsum.tile([U, ZS], mybir.dt.float32, tag="warm", bufs=1)
                    for i in range(warmup):
                        nc.tensor.matmul(
                            wt[:],
                            warm_sb[:, 0:U],
                            warm_sb[:],
                            start=(i == 0),
                            stop=(i == warmup - 1),
                        )
                    ws = tmp_pool.tile(
                        [U, ZS], mybir.dt.float32, tag="warms", bufs=1
                    )
                    nc.scalar.copy(out=ws[:], in_=wt[:])
                    nc.sync.dma_start(out=junk_d[:], in_=ws[:])
                for s in range(S):
                    pt = gpsum.tile([U, ZS], mybir.dt.float32, tag="gps")
                    nc.tensor.matmul(
                        pt[:],
                        x0_sb[:, s * U : (s + 1) * U],
                        oh_sb[:],
                        start=True,
                        stop=True,
                    )
                    nc.scalar.copy(out=seg(x0g, s), in_=pt[:])

            slab_state = {"idx": -1, "tile": None}
            for g in groups:
                sos, builds, slot_of, path_ops = (
                    g["sos"],
                    g["builds"],
                    g["slot_of"],
                    g["path_ops"],
                )
                n_slots_g = g["n_slots"]

                def ref_space(r):
                    if r[0] in ("sg", "pair"):
                        return "prod"
                    return r[0]

                def ref_pos(r):
                    if r[0] in ("sg", "pair"):
                        return slot_of[r]
                    if r[0] == "x1":
                        return posmap[r[1]]
                    return r[1]

                CAPST = {"prod": MAXSTRIDE, "x1": S - 1, "x0g": S - 1}

                # ---- unify build runs into generic DVE ops ----
                # member = (in0_ref, in1_ref, out_slot)
                def bmembers(b):
                    if b[0] == "sg_run":
                        _, s0, s_lo, n, slot_lo = b
                        return [
                            (("x0g", s0), ("x1s", s_lo + i), slot_lo + i)
                            for i in range(n)
                        ]
                    _, delta, a_lo, n, slot_lo = b
                    return [
                        (("x1s", a_lo + i), ("x1s", a_lo + delta + i), slot_lo + i)
                        for i in range(n)
                    ]

                def mref_space(r):
                    return "x1" if r[0] == "x1s" else ref_space(r)

                def mref_pos(r):
                    return posmap[r[1]] if r[0] == "x1s" else ref_pos(r)

                def match2(A, B, out_fixed):
                    orders = [(A, B), (B, A)]
                    if out_fixed:
                        orders = [(A, B)] if A[2] < B[2] else [(B, A)]
                    for X, Y in orders:
                        if out_fixed and not (0 < Y[2] - X[2] <= MAXSTRIDE):
                            continue
                        for xa, xb in ((X[0], X[1]), (X[1], X[0])):
                            for ya, yb in ((Y[0], Y[1]), (Y[1], Y[0])):
                                if mref_space(xa) != mref_space(ya):
                                    continue
                                if mref_space(xb) != mref_space(yb):
                                    continue
                                s0_ = mref_pos(ya) - mref_pos(xa)
                                s1_ = mref_pos(yb) - mref_pos(xb)
                                if s0_ < 0 or s1_ < 0 or (s0_ == 0 and s1_ == 0):
                                    continue
                                if s0_ > CAPST[mref_space(xa)]:
                                    continue
                                if s1_ > CAPST[mref_space(xb)]:
                                    continue
                                return [(xa, xb, X[2]), (ya, yb, Y[2])]
                    return None

                def run_ok(ms):
                    pa = [mref_pos(m[0]) for m in ms]
                    pb = [mref_pos(m[1]) for m in ms]
                    po = [m[2] for m in ms]
                    da = {pa[i + 1] - pa[i] for i in range(len(ms) - 1)}
                    db = {pb[i + 1] - pb[i] for i in range(len(ms) - 1)}
                    do = {po[i + 1] - po[i] for i in range(len(ms) - 1)}
                    if len(da) != 1 or len(db) != 1 or len(do) != 1:
                        return False
                    sa, sb, so_ = da.pop(), db.pop(), do.pop()
                    if sa < 0 or sb < 0 or (sa == 0 and sb == 0):
                        return False
                    if not (0 < so_ <= MAXSTRIDE):
                        return False
                    if sa > CAPST[mref_space(ms[0][0])]:
                        return False
                    if sb > CAPST[mref_space(ms[0][1])]:
                        return False
                    return True

                build_ops = []  # (order_key, [members])
                singles_b = []
                for bi, b in enumerate(builds):
                    ms = bmembers(b)
                    if len(ms) >= 2 and run_ok(ms):
                        build_ops.append((bi, ms))
                    else:
                        for m in ms:
                            singles_b.append((bi, m))
                usedb = [False] * len(singles_b)
                for i in range(len(singles_b)):
                    if usedb[i]:
                        continue
                    for j in range(i + 1, len(singles_b)):
                        if usedb[j]:
                            continue
                        m = match2(singles_b[i][1], singles_b[j][1], True)
                        if m:
                            build_ops.append((singles_b[i][0], m))
                            usedb[i] = usedb[j] = True
                            break
                    if not usedb[i]:
                        build_ops.append((singles_b[i][0], [singles_b[i][1]]))
                        usedb[i] = True
                build_ops.sort(key=lambda x: x[0])
                build_ops = [ms for _, ms in build_ops]

                slot_done_at = {}
                for bi, ms in enumerate(build_ops):
                    for m in ms:
                        slot_done_at[m[2]] = bi

                def space_pos(r):
                    if r[0] in ("sg", "pair"):
                        return ("prod", slot_of[r])
                    if r[0] == "x1":
                        return ("x1", posmap[r[1]])
                    return ("x0g", r[1])

                # ---- greedy merged-final planning ----
                # run instr: in0 broadcast ref, partners at (space, lo, stride)
                finals = [i for i, po in enumerate(path_ops) if po[0] >= 2]
                d1s = [i for i, po in enumerate(path_ops) if po[0] == 1]
                unsched = set(finals)
                instrs = []  # (in0_desc, in1_desc, member_paths)
                # desc = ("bcast", ref) | ("run", space, lo, stride)
                while unsched:
                    cand_groups = {}
                    for i in unsched:
                        d, r1, r2, c, so = path_ops[i]
                        for rb, rp in ((r1, r2), (r2, r1)):
                            sp, pos = space_pos(rp)
                            cand_groups.setdefault((rb, sp), {}).setdefault(pos, i)
                    best = None
                    for (rb, sp), pmp in cand_groups.items():
                        ps = sorted(pmp)
                        for ai in range(len(ps)):
                            for bi2 in range(ai + 1, len(ps)):
                                st = ps[bi2] - ps[ai]
                                if st > CAPST[sp]:
                                    break
                                run = [ps[ai], ps[bi2]]
                                nxt = ps[bi2] + st
                                while nxt in pmp and len(run) < MAXRUN:
                                    run.append(nxt)
                                    nxt += st
                                if best is None or len(run) > best[0]:
                                    best = (
                                        len(run),
                                        (rb, sp, run[0], st, [pmp[p] for p in run]),
                                    )
                                if best[0] >= MAXRUN:
                                    break
                            if best is not None and best[0] >= MAXRUN:
                                break
                        if best is not None and best[0] >= MAXRUN:
                            break
                    if best is None or best[0] < 3:
                        break
                    _, (rb, sp, lo, st, members) = best
                    instrs.append((("bcast", rb), ("run", sp, lo, st), members))
                    unsched -= set(members)
                # pair up the leftovers (any two compatible finals share one op)
                leftover = sorted(unsched)
                usedf = [False] * len(leftover)
                for ii in range(len(leftover)):
                    if usedf[ii]:
                        continue
                    i = leftover[ii]
                    d, r1, r2, c, so = path_ops[i]
                    A = (r1, r2, i)
                    matched = False
                    for jj in range(ii + 1, len(leftover)):
                        if usedf[jj]:
                            continue
                        jp = leftover[jj]
                        d2, q1, q2, c2, so2 = path_ops[jp]
                        m = match2((r1, r2, i), (q1, q2, jp), False)
                        if m:
                            (xa, xb, pi), (ya, yb, pj) = m
                            st0 = ref_pos(ya) - ref_pos(xa)
                            st1 = ref_pos(yb) - ref_pos(xb)
                            d0 = (
                                ("bcast", xa)
                                if st0 == 0
                                else ("run", ref_space(xa), ref_pos(xa), st0)
                            )
                            d1_ = (
                                ("bcast", xb)
                                if st1 == 0
                                else ("run", ref_space(xb), ref_pos(xb), st1)
                            )
                            instrs.append((d0, d1_, [pi, pj]))
                            usedf[ii] = usedf[jj] = True
                            matched = True
                            break
                    if not matched:
                        sp, pos = space_pos(r2)
                        instrs.append((("bcast", r1), ("run", sp, pos, 1), [i]))
                        usedf[ii] = True

                def desc_ready(desc):
                    kind = desc[0]
                    if kind == "bcast":
                        r = desc[1]
                        if r[0] in ("sg", "pair"):
                            return slot_done_at.get(slot_of[r], -1)
                        return -1
                    _, sp, lo, st = desc
                    return -1  # positions handled by caller for prod

                def instr_ready(ins):
                    d0, d1_, members = ins
                    bi = -1
                    for desc, nmem in ((d0, len(members)), (d1_, len(members))):
                        if desc[0] == "bcast":
                            r = desc[1]
                            if r[0] in ("sg", "pair"):
                                bi = max(bi, slot_done_at.get(slot_of[r], -1))
                        else:
                            _, sp, lo, st = desc
                            if sp == "prod":
                                for k in range(nmem):
                                    bi = max(
                                        bi, slot_done_at.get(lo + k * st, -1)
                                    )
                    return bi

                ready_after = defaultdict(list)  # build-op idx -> events
                for i in d1s:
                    r1 = path_ops[i][1]
                    ready_after[slot_done_at.get(slot_of[r1], -1)].append(("d1", i))
                for j, ins in enumerate(instrs):
                    ready_after[instr_ready(ins)].append(("ins", j))

                # dry pass: MM emission order -> start/stop flags per so
                mm_seq = []
                for bi in range(-1, len(build_ops)):
                    for kind, j in ready_after[bi]:
                        if kind == "d1":
                            mm_seq.append(j)
                        else:
                            mm_seq.extend(instrs[j][2])
                first_for_so = {}
                last_for_so = {}
                for i in mm_seq:
                    so = path_ops[i][4]
                    if so not in first_for_so:
                        first_for_so[so] = i
                    last_for_so[so] = i
                assert len(mm_seq) == len(path_ops)

                acc = {}
                with tc.tile_pool(
                    name=f"acc{sos[0]}", bufs=8, space="PSUM"
                ) as acc_pool:
                    for so in sos:
                        if so in first_for_so:
                            acc[so] = acc_pool.tile(
                                [U, ZS],
                                mybir.dt.float32,
                                tag=f"acc{sos.index(so)}",
                                name=f"acc_{so}",
                                bufs=1,
                            )

                    base_of = {
                        "prod": (prod, max_slots),
                        "x1": (x1t, S),
                        "x0g": (x0g, S),
                    }

                    def pref(r):
                        kind, key = r
                        if kind in ("x1", "x1s"):
                            return seg(x1t, posmap[key])
                        if kind == "x0g":
                            return seg(x0g, key)
                        return seg(prod, slot_of[r])

                    def ap_of(desc, n):
                        if desc[0] == "bcast":
                            a = pref(desc[1])
                            if n == 1:
                                return a
                            return a.rearrange("p (o z) -> p o z", o=1).broadcast_to(
                                [U, n, ZS]
                            )
                        _, sp, lo, st = desc
                        base, W = base_of[sp]
                        if n == 1:
                            return seg(base, lo)
                        base3 = base[:].rearrange("p (w z) -> p w z", w=W)
                        if st > 0:
                            return base3[:, lo : lo + (n - 1) * st + 1 : st, :]
                        end = lo + (n - 1) * st - 1
                        if end < 0:
                            return base3[:, lo :: st, :][:, 0:n, :]
                        return base3[:, lo : end : st, :]

                    def emit_mm(i, rhs):
                        d, r1, r2, c, so = path_ops[i]
                        gi = len(coeff_order)
                        coeff_order.append(c)
                        sj, sk = gi // SLAB, gi % SLAB
                        if slab_state["idx"] != sj:
                            slab_state["idx"] = sj
                            stt = slab_pool.tile(
                                [U, SLAB * U], dt, tag="slab", name=f"slab{sj}"
                            )
                            slab_state["tile"] = stt
                            nc.scalar.dma_start(
                                out=stt[:].rearrange("p (d c) -> p d c", d=SLAB),
                                in_=cd_d[sj * SLAB * U : (sj + 1) * SLAB * U, :]
                                .rearrange("(d p) c -> p d c", p=U),
                            )
                        stt = slab_state["tile"]
                        nc.tensor.matmul(
                            acc[so][:],
                            stt[:, sk * U : (sk + 1) * U],
                            rhs,
                            start=(i == first_for_so[so]),
                            stop=(i == last_for_so[so]),
                        )

                    def emit_instr(j):
                        d0, d1_, members = instrs[j]
                        n = len(members)
                        t1 = tmp_pool.tile(
                            [U, n * ZS], dt, tag=f"tmp{n}", bufs=TMP_BUFS[n],
                            name=f"t{sos[0]}_{j}",
                        )
                        out = t1[:]
                        if n > 1:
                            out = out.rearrange("p (r z) -> p r z", r=n)
                        nc.vector.tensor_tensor(
                            out=out, in0=ap_of(d0, n), in1=ap_of(d1_, n), op=MULT
                        )
                        for k, i in enumerate(members):
                            emit_mm(i, t1[:, k * ZS : (k + 1) * ZS])

                    def emit_event(ev):
                        kind, j = ev
                        if kind == "d1":
                            emit_mm(j, pref(path_ops[j][1]))
                        else:
                            emit_instr(j)

                    def emit_build(ms):
                        n = len(ms)
                        if n == 1:
                            (ra, rb_, o) = ms[0]
                            nc.vector.tensor_tensor(
                                out=seg(prod, o), in0=pref(ra), in1=pref(rb_),
                                op=MULT,
                            )
                            return
                        # derive strides from members (run or matched pair)
                        (a0, b0, o0), (a1, b1, o1) = ms[0], ms[1]
                        st_a = mref_pos(a1) - mref_pos(a0)
                        st_b = mref_pos(b1) - mref_pos(b0)
                        st_o = o1 - o0
                        da = (
                            ("bcast", a0)
                            if st_a == 0
                            else ("run", mref_space(a0), mref_pos(a0), st_a)
                        )
                        db = (
                            ("bcast", b0)
                            if st_b == 0
                            else ("run", mref_space(b0), mref_pos(b0), st_b)
                        )
                        base3 = prod[:].rearrange("p (w z) -> p w z", w=max_slots)
                        out = base3[:, o0 : o0 + (n - 1) * st_o + 1 : st_o, :]
                        nc.vector.tensor_tensor(
                            out=out, in0=ap_of(da, n), in1=ap_of(db, n), op=MULT
                        )

                    for ev in ready_after[-1]:
                        emit_event(ev)
                    for bi, ms in enumerate(build_ops):
                        emit_build(ms)
                        for ev in ready_after[bi]:
                            emit_event(ev)

                    # drain each accumulator through a small SBUF stage,
                    # splitting copies across ScalarE and DVE and the DMAs
                    # across both HWDGE queues to shorten the tail
                    for k, so in enumerate(sos):
                        assert so in acc, f"output segment {so} has no paths"
                        ostg = tmp_pool.tile(
                            [U, ZS], dt, tag="ostg", bufs=4, name=f"ostg{so}"
                        )
                        if k % 2 == 0:
                            nc.scalar.copy(out=ostg[:], in_=acc[so][:])
                        else:
                            nc.vector.tensor_copy(out=ostg[:], in_=acc[so][:])
                        deng = nc.sync if k % 2 == 0 else nc.scalar
                        deng.dma_start(
                            out=out_d[so * U : (so + 1) * U, :], in_=ostg[:]
                        )

    nc.compile()
    return nc, coeff_order, seg_order


def kernel(x0, x1, coeff1, coeff2, coeff3, i0, idx1, idx2, idx3):
    global LAST_EXEC_NS, LAST_RESULTS
    from concourse.bass_utils import run_bass_kernel_spmd

    x0 = np.asarray(x0, dtype=np.float32)
    x1 = np.asarray(x1, dtype=np.float32)
    i0 = np.asarray(i0).astype(np.int64)
    idxs = [np.asarray(a) for a in (idx1, idx2, idx3)]
    coeffs = [np.asarray(c, dtype=np.float32) for c in (coeff1, coeff2, coeff3)]

    dtype_name = os.environ.get("KERNEL_DTYPE", "bfloat16")
    act_frac = float(os.environ.get("KERNEL_ACT_FRAC", "0.55"))
    pool_frac = float(os.environ.get("KERNEL_POOL_FRAC", "0.3"))
    warmup = int(os.environ.get("KERNEL_WARMUP", "12"))
    gpsimd_every = int(os.environ.get("KERNEL_GPSIMD_EVERY", "0"))
    npdt = np.float32
    if dtype_name != F32:
        import ml_dtypes

        npdt = ml_dtypes.bfloat16

    groups, all_sq = _build_plan(idxs, coeffs)
    nc, coeff_order, seg_order = _build_bass(groups, dtype_name, act_frac, warmup, pool_frac, all_sq, gpsimd_every)
    n_slabs = (len(coeff_order) + SLAB - 1) // SLAB
    cdiag = np.zeros((n_slabs * SLAB * U, U), dtype=npdt)
    for gi, c in enumerate(coeff_order):
        blk = cdiag[gi * U : (gi + 1) * U, :]
        np.fill_diagonal(blk, np.asarray(c, dtype=npdt))

    in_maps = []
    eye = np.arange(NELEM)
    x0c = x0.astype(npdt)
    for c in range(NCORES):
        zl, zh = c * ZS, (c + 1) * ZS
        shard = x1[zl:zh]
        x1t = np.ascontiguousarray(
            shard.reshape(ZS, S, U).transpose(1, 2, 0).reshape(S * U, ZS)
        ).astype(npdt)
        oh = (i0[zl:zh][None, :] == eye[:, None]).astype(npdt)
        in_maps.append({"x1t": x1t, "x0w": x0c, "oh": oh, "cdiag": cdiag})

    trace = os.environ.get("BASS_TRACE", "") not in ("", "0")
    trace_cores = None
    tc_env = os.environ.get("KERNEL_TRACE_CORES", "")
    if tc_env:
        trace_cores = [int(x) for x in tc_env.split(",")]
    res = run_bass_kernel_spmd(
        nc, in_maps, core_ids=list(range(NCORES)), trace=trace,
        trace_cores=trace_cores,
    )
    LAST_EXEC_NS = res.exec_time_ns
    LAST_RESULTS = res

    out = np.empty((Z, S * U), dtype=np.float32)
    for c in range(NCORES):
        outt = np.asarray(res.results[c]["outt"], dtype=np.float32)
        out[c * ZS : (c + 1) * ZS] = (
            outt.reshape(S, U, ZS).transpose(2, 0, 1).reshape(ZS, S * U)
        )
    return out



# revision 24
# speedup vs baseline: 1.0030x; 1.0030x over previous
"""Trainium2 Bass kernel for a segmented tensor-product contraction.

Computation (per batch row z, channel u, segments of width U=128):
  out[z, so, u] += c_p * x0[i0[z], s0_p, u] * prod_k x1[z, sk_p, u]
for 256 paths of degree 1..3 over S=16 segments.

Strategy:
  - Data-parallel over z across 8 NeuronCores (512 rows each).
  - On-chip layout: [u (partitions) x z (free dim)] per segment; every
    elementwise op is a [128, 512] instruction.
  - x0 row gather: host builds one-hot(i0) per core; TensorEngine computes
    x0gT[s] = x0[:, s]^T @ onehot (gather + transpose for free).
  - Factorization (globally optimized per so-group): suffix products
    sg(s0,s) = x0g[s0]*x1[s] and pairs pr(a,b) = x1[a]*x1[b]; each path is
    one tensor_tensor plus a coefficient scale on ScalarE (or a fused
    scalar_tensor_tensor on VectorE for a fraction of paths).
  - Product builds are packed into merged multi-segment instructions
    (sg runs share one instr via a stride-0 broadcast of x0g[s0]; pair
    runs along constant delta read contiguous x1 spans).
  - Output accumulation on TensorEngine: identity-matmul each path term
    into a per-segment PSUM bank (exact f32 adds). 16 output segments ->
    two groups of 8 banks; the so-partition is optimized to minimize
    duplicated product builds.
"""

import os
from collections import defaultdict

import numpy as np

U = 128
S = 16
NELEM = 64
Z = 4096
NCORES = 8
ZS = Z // NCORES  # 512 rows per core

LAST_EXEC_NS = None
LAST_RESULTS = None

F32 = "float32"


def _parse_paths(idxs, coeffs):
    paths = []  # (degree, x1segs_sorted, s0, so, coeff)
    for idx, cf in zip(idxs, coeffs):
        d = idx.shape[1] - 2
        for r, c in zip(idx, cf):
            r = [int(v) for v in r]
            paths.append((d, tuple(sorted(r[:d])), r[d], r[d + 1], float(c)))
    return paths


def _options(p):
    """Candidate (products, form) decompositions for a path.

    Each option: (frozenset of product keys, form)
    form = (in0_ref, in1_ref) with refs ('x1',s) ('x0g',s) ('sg',(s0,s))
    ('pair',(a,b)); d1 form = (('sg',(s0,s)), None).
    Product keys: ('sg',(s0,s)), ('pair',(a,b)).
    """
    d, segs, s0, so, c = p
    if d == 1:
        k = ("sg", (s0, segs[0]))
        return [(frozenset([k]), (k, None))]
    if d == 2:
        a, b = segs
        opts = [
            (frozenset([("sg", (s0, b))]), (("x1", a), ("sg", (s0, b)))),
            (frozenset([("sg", (s0, a))]), (("x1", b), ("sg", (s0, a)))),
            (frozenset([("pair", (a, b))]), (("pair", (a, b)), ("x0g", s0))),
        ]
        return opts
    a, b, cc = segs
    return [
        (
            frozenset([("pair", (a, b)), ("sg", (s0, cc))]),
            (("pair", (a, b)), ("sg", (s0, cc))),
        ),
        (
            frozenset([("pair", (a, cc)), ("sg", (s0, b))]),
            (("pair", (a, cc)), ("sg", (s0, b))),
        ),
        (
            frozenset([("pair", (b, cc)), ("sg", (s0, a))]),
            (("pair", (b, cc)), ("sg", (s0, a))),
        ),
    ]


def _optimize_group(gpaths, n_sweeps=4):
    """Choose per-path decomposition minimizing total unique products."""
    choices = [0] * len(gpaths)
    opts = [_options(p) for p in gpaths]
    for _ in range(n_sweeps):
        counts = defaultdict(int)
        for i, p in enumerate(gpaths):
            for k in opts[i][choices[i]][0]:
                counts[k] += 1
        changed = False
        for i, p in enumerate(gpaths):
            best, best_cost = choices[i], None
            for j, (prods, _) in enumerate(opts[i]):
                # marginal cost: products not used by anyone else
                cost = 0.0
                for k in prods:
                    others = counts[k] - (1 if k in opts[i][choices[i]][0] else 0)
                    cost += 1.0 / (1 + others)
                if best_cost is None or cost < best_cost - 1e-9:
                    best, best_cost = j, cost
            if best != choices[i]:
                # update counts incrementally
                for k in opts[i][choices[i]][0]:
                    counts[k] -= 1
                for k in opts[i][best][0]:
                    counts[k] += 1
                choices[i] = best
                changed = True
        if not changed:
            break
    products = set()
    forms = []
    for i, p in enumerate(gpaths):
        prods, form = opts[i][choices[i]]
        products |= prods
        forms.append(form)
    return products, forms


def _group_cost(paths, sos_a):
    """Estimate total builds for a candidate so-partition."""
    total = 0
    for sos in (sos_a, [s for s in range(S) if s not in sos_a]):
        gp = [p for p in paths if p[3] in sos]
        prods, _ = _optimize_group(gp, n_sweeps=4)
        total += len(prods)
    return total


def _optimize_partition(paths):
    """Two-stage exhaustive search of the 8/8 so-partition (C(16,8)/2 =
    6435 candidates): cheap 1-sweep proxy scan, then exact re-scoring of
    the best candidates."""
    from itertools import combinations

    def cost(sos_a, sweeps):
        total = 0
        for sos in (sos_a, [s for s in range(S) if s not in sos_a]):
            gp = [p for p in paths if p[3] in sos]
            prods, _ = _optimize_group(gp, n_sweeps=sweeps)
            total += len(prods)
        return total

    cands = [c for c in combinations(range(S), 8) if 0 in c]
    scored = sorted(cands, key=lambda c: cost(list(c), 1))[:30]
    best = min(scored, key=lambda c: cost(list(c), 4))
    cur = list(best)
    other = [s for s in range(S) if s not in cur]
    return cur, other


def _plan_merges(products):
    """Pack product builds into merged instructions.

    Returns (slot_of, builds) where slot_of maps product key -> slot index
    and builds is a list of ('sg_run', s0, s_lo, n, slot_lo) or
    ('pair_run', delta, a_lo, n, slot_lo).
    """
    slot_of = {}
    builds = []
    next_slot = 0
    sgs = defaultdict(list)  # s0 -> sorted s list
    prs = defaultdict(list)  # delta -> sorted a list
    for k in products:
        if k[0] == "sg":
            sgs[k[1][0]].append(k[1][1])
        else:
            a, b = k[1]
            prs[b - a].append(a)
    squares = sorted(prs.pop(0, []))
    # pair runs first: they depend only on x1t (no gather chain), so the
    # DVE can start on them while the x0 gather pipeline fills
    for delta in sorted(prs):
        aa = sorted(prs[delta])
        run = [aa[0]]
        for a in aa[1:] + [None]:
            if a is not None and a == run[-1] + 1:
                run.append(a)
            else:
                kind = "pair_run"
                builds.append((kind, delta, run[0], len(run), next_slot))
                for i, ra in enumerate(run):
                    slot_of[("pair", (ra, ra + delta))] = next_slot + i
                next_slot += len(run)
                if a is not None:
                    run = [a]
    for s0 in sorted(sgs):
        ss = sorted(sgs[s0])
        run = [ss[0]]
        for s in ss[1:] + [None]:
            if s is not None and s == run[-1] + 1:
                run.append(s)
            else:
                builds.append(("sg_run", s0, run[0], len(run), next_slot))
                for i, rs in enumerate(run):
                    slot_of[("sg", (s0, rs))] = next_slot + i
                next_slot += len(run)
                if s is not None:
                    run = [s]
    return slot_of, builds, next_slot, squares


def _build_plan(idxs, coeffs):
    """Full schedule. Returns (groups, all_sq).

    Joint factorization over ALL paths; products used by both so-groups
    are built once into a shared slot region and stay resident across
    both PSUM phases. Group-unique products overlay one reuse region.
    """
    paths = _parse_paths(idxs, coeffs)
    products, forms = _optimize_group(paths, n_sweeps=6)
    part_a = list(range(8))
    part_b = list(range(8, 16))

    all_sq = sorted(
        set(k[1][0] for k in products if k[0] == "pair" and k[1][0] == k[1][1])
    )
    sq_keys = set(("pair", (s, s)) for s in all_sq)

    # classify products by which groups use them
    use_a, use_b = set(), set()
    for p, form in zip(paths, forms):
        tgt = use_a if p[3] in part_a else use_b
        for r in form:
            if r and r[0] in ("sg", "pair") and r not in sq_keys:
                tgt.add(r)
    shared = use_a & use_b
    uniq = {0: use_a - shared, 1: use_b - shared}

    slot_shared, builds_shared, ns, _ = _plan_merges(shared)
    slot_a, builds_a, na, _ = _plan_merges(uniq[0])
    slot_b, builds_b, nb, _ = _plan_merges(uniq[1])
    base2 = ns
    n_main = ns + max(na, nb)
    sq_slot = {s: n_main + i for i, s in enumerate(all_sq)}
    n_slots = n_main + len(all_sq)

    def shift(builds, slot, delta):
        bs = [(b[0], b[1], b[2], b[3], b[4] + delta) for b in builds]
        sl = {k: v + delta for k, v in slot.items()}
        return bs, sl

    builds_a, slot_a = shift(builds_a, slot_a, base2)
    builds_b, slot_b = shift(builds_b, slot_b, base2)

    groups = []
    for gi, (sos, gbuilds, gslot) in enumerate(
        (
            (part_a, builds_shared + builds_a, {**slot_shared, **slot_a}),
            (part_b, builds_b, {**slot_shared, **slot_b}),
        )
    ):
        slot_of = dict(gslot)
        for s in all_sq:
            slot_of[("pair", (s, s))] = sq_slot[s]
        gidx = [i for i, p in enumerate(paths) if p[3] in sos]
        order = sorted(
            gidx,
            key=lambda i: (
                paths[i][0] != 1,
                max(
                    (
                        slot_of[r]
                        for r in forms[i]
                        if r and r[0] in ("sg", "pair")
                    ),
                    default=-1,
                ),
            ),
        )
        path_ops = [
            (paths[i][0], forms[i][0], forms[i][1], paths[i][4], paths[i][3])
            for i in order
        ]
        groups.append(
            dict(
                sos=sos,
                builds=gbuilds,
                slot_of=slot_of,
                n_slots=n_slots,
                path_ops=path_ops,
            )
        )
    return groups, all_sq


SLAB = 32  # coefficient-diagonal matrices per DMA slab


def _build_bass(groups, dtype_name, act_frac, warmup, pool_frac=0.0, all_sq=(), gpsimd_every=0):
    import concourse.bacc as bacc
    import concourse.mybir as mybir
    from concourse.tile import TileContext

    dt = mybir.dt.float32 if dtype_name == F32 else mybir.dt.bfloat16
    MULT = mybir.AluOpType.mult

    nc = bacc.Bacc("TRN2", debug=False)

    n_paths_total = sum(len(g["path_ops"]) for g in groups)
    n_slabs = (n_paths_total + SLAB - 1) // SLAB

    x1t_d = nc.dram_tensor("x1t", [S * U, ZS], dt, kind="ExternalInput")
    x0_d = nc.dram_tensor("x0w", [NELEM, S * U], dt, kind="ExternalInput")
    oh_d = nc.dram_tensor("oh", [NELEM, ZS], dt, kind="ExternalInput")
    cd_d = nc.dram_tensor("cdiag", [n_slabs * SLAB * U, U], dt, kind="ExternalInput")
    out_d = nc.dram_tensor("outt", [S * U, ZS], dt, kind="ExternalOutput")
    junk_d = nc.dram_tensor("junk", [U, ZS], mybir.dt.float32)

    max_slots = max(g["n_slots"] for g in groups)
    coeff_order = []  # flat list of coefficients in emission order

    MAXRUN = 4
    MAXSTRIDE = 63  # ISA: 16-bit step_elem field caps r-stride at 32767 elems
    TMP_BUFS = {1: 10, 2: 4, 3: 2, 4: 3}

    with TileContext(nc) as tc:
        with tc.tile_pool(name="persist", bufs=1) as persist, tc.tile_pool(
            name="tmp", bufs=6
        ) as tmp_pool, tc.tile_pool(name="slab", bufs=2) as slab_pool:
            x1t = persist.tile([U, S * ZS], dt, tag="x1t")
            x0g = persist.tile([U, S * ZS], dt, tag="x0g")
            prod = persist.tile([U, max_slots * ZS], dt, tag="prod")
            x0_sb = persist.tile([NELEM, S * U], dt, tag="x0w")
            oh_sb = persist.tile([NELEM, ZS], dt, tag="oh")
            warm_sb = persist.tile([NELEM, ZS], dt, tag="warmsrc")

            def seg(t, s):
                return t[:, s * ZS : (s + 1) * ZS]

            def span(t, lo, n):
                return t[:, lo * ZS : (lo + n) * ZS]

            # x1t DRAM rows are host-permuted into first-use order
            # (seg_order); SBUF position k holds segment seg_order[k]. DMA in
            # four contiguous 4-segment chunks on one queue (cross-queue DMAs
            # into one tile serialize on a WAW dependency).
            seg_order = []

            def _want(s):
                if s not in seg_order:
                    seg_order.append(s)

            for g in groups:
                for b in g["builds"]:
                    kind, key, lo, n, _ = b
                    if kind == "pair_run":
                        for i in range(n):
                            _want(lo + i)
                            _want(lo + i + key)
                    else:
                        for i in range(n):
                            _want(lo + i)
            for s in all_sq:
                _want(s)
            for s in range(S):
                _want(s)
            posmap = {s: s for s in range(S)}  # identity: natural DRAM order
            for s in seg_order[:2]:
                nc.sync.dma_start(
                    out=seg(x1t, s), in_=x1t_d[s * U : (s + 1) * U, :]
                )
            nc.scalar.dma_start(out=oh_sb[:], in_=oh_d[:])
            nc.scalar.dma_start(out=x0_sb[:], in_=x0_d[:])
            for k, s in enumerate(seg_order[2:]):
                eng = nc.sync if k % 2 == 0 else nc.scalar
                eng.dma_start(
                    out=seg(x1t, s), in_=x1t_d[s * U : (s + 1) * U, :]
                )

            # global square products on ScalarE (before any ACT Copy use,
            # emitted as consecutive runs to avoid table-set thrashing)
            if all_sq:
                max_g = groups[0]["n_slots"] - len(all_sq)
                for ri, s in enumerate(all_sq):
                    nc.scalar.activation(
                        span(prod, max_g + ri, 1),
                        seg(x1t, posmap[s]),
                        mybir.ActivationFunctionType.Square,
                    )

            nc.gpsimd.memset(warm_sb[:], 0.0)
            # PE warmup burst (reads a memset tile: no DMA dependency, keeps
            # the HAM clock-gate warm before the real stream starts)
            # + the 16 x0 gather matmuls
            with tc.tile_pool(name="gpsum", bufs=4, space="PSUM") as gpsum:
                if warmup > 0:
                    wt = gpsum.tile([U, ZS], mybir.dt.float32, tag="warm", bufs=1)
                    for i in range(warmup):
                        nc.tensor.matmul(
                            wt[:],
                            warm_sb[:, 0:U],
                            warm_sb[:],
                            start=(i == 0),
                            stop=(i == warmup - 1),
                        )
                    ws = tmp_pool.tile(
                        [U, ZS], mybir.dt.float32, tag="warms", bufs=1
                    )
                    nc.scalar.copy(out=ws[:], in_=wt[:])
                    nc.sync.dma_start(out=junk_d[:], in_=ws[:])
                for s in range(S):
                    pt = gpsum.tile([U, ZS], mybir.dt.float32, tag="gps")
                    nc.tensor.matmul(
                        pt[:],
                        x0_sb[:, s * U : (s + 1) * U],
                        oh_sb[:],
                        start=True,
                        stop=True,
                    )
                    nc.scalar.copy(out=seg(x0g, s), in_=pt[:])

            slab_state = {"idx": -1, "tile": None}
            for g in groups:
                sos, builds, slot_of, path_ops = (
                    g["sos"],
                    g["builds"],
                    g["slot_of"],
                    g["path_ops"],
                )
                n_slots_g = g["n_slots"]

                def ref_space(r):
                    if r[0] in ("sg", "pair"):
                        return "prod"
                    return r[0]

                def ref_pos(r):
                    if r[0] in ("sg", "pair"):
                        return slot_of[r]
                    if r[0] == "x1":
                        return posmap[r[1]]
                    return r[1]

                CAPST = {"prod": MAXSTRIDE, "x1": S - 1, "x0g": S - 1}

                # ---- unify build runs into generic DVE ops ----
                # member = (in0_ref, in1_ref, out_slot)
                def bmembers(b):
                    if b[0] == "sg_run":
                        _, s0, s_lo, n, slot_lo = b
                        return [
                            (("x0g", s0), ("x1s", s_lo + i), slot_lo + i)
                            for i in range(n)
                        ]
                    _, delta, a_lo, n, slot_lo = b
                    return [
                        (("x1s", a_lo + i), ("x1s", a_lo + delta + i), slot_lo + i)
                        for i in range(n)
                    ]

                def mref_space(r):
                    return "x1" if r[0] == "x1s" else ref_space(r)

                def mref_pos(r):
                    return posmap[r[1]] if r[0] == "x1s" else ref_pos(r)

                def match2(A, B, out_fixed):
                    orders = [(A, B), (B, A)]
                    if out_fixed:
                        orders = [(A, B)] if A[2] < B[2] else [(B, A)]
                    for X, Y in orders:
                        if out_fixed and not (0 < Y[2] - X[2] <= MAXSTRIDE):
                            continue
                        for xa, xb in ((X[0], X[1]), (X[1], X[0])):
                            for ya, yb in ((Y[0], Y[1]), (Y[1], Y[0])):
                                if mref_space(xa) != mref_space(ya):
                                    continue
                                if mref_space(xb) != mref_space(yb):
                                    continue
                                s0_ = mref_pos(ya) - mref_pos(xa)
                                s1_ = mref_pos(yb) - mref_pos(xb)
                                if s0_ < 0 or s1_ < 0 or (s0_ == 0 and s1_ == 0):
                                    continue
                                if s0_ > CAPST[mref_space(xa)]:
                                    continue
                                if s1_ > CAPST[mref_space(xb)]:
                                    continue
                                return [(xa, xb, X[2]), (ya, yb, Y[2])]
                    return None

                def run_ok(ms):
                    pa = [mref_pos(m[0]) for m in ms]
                    pb = [mref_pos(m[1]) for m in ms]
                    po = [m[2] for m in ms]
                    da = {pa[i + 1] - pa[i] for i in range(len(ms) - 1)}
                    db = {pb[i + 1] - pb[i] for i in range(len(ms) - 1)}
                    do = {po[i + 1] - po[i] for i in range(len(ms) - 1)}
                    if len(da) != 1 or len(db) != 1 or len(do) != 1:
                        return False
                    sa, sb, so_ = da.pop(), db.pop(), do.pop()
                    if sa < 0 or sb < 0 or (sa == 0 and sb == 0):
                        return False
                    if not (0 < so_ <= MAXSTRIDE):
                        return False
                    if sa > CAPST[mref_space(ms[0][0])]:
                        return False
                    if sb > CAPST[mref_space(ms[0][1])]:
                        return False
                    return True

                build_ops = []  # (order_key, [members])
                singles_b = []
                for bi, b in enumerate(builds):
                    ms = bmembers(b)
                    if len(ms) >= 2 and run_ok(ms):
                        build_ops.append((bi, ms))
                    else:
                        for m in ms:
                            singles_b.append((bi, m))
                usedb = [False] * len(singles_b)
                for i in range(len(singles_b)):
                    if usedb[i]:
                        continue
                    for j in range(i + 1, len(singles_b)):
                        if usedb[j]:
                            continue
                        m = match2(singles_b[i][1], singles_b[j][1], True)
                        if m:
                            build_ops.append((singles_b[i][0], m))
                            usedb[i] = usedb[j] = True
                            break
                    if not usedb[i]:
                        build_ops.append((singles_b[i][0], [singles_b[i][1]]))
                        usedb[i] = True
                build_ops.sort(key=lambda x: x[0])
                build_ops = [ms for _, ms in build_ops]

                slot_done_at = {}
                for bi, ms in enumerate(build_ops):
                    for m in ms:
                        slot_done_at[m[2]] = bi

                def space_pos(r):
                    if r[0] in ("sg", "pair"):
                        return ("prod", slot_of[r])
                    if r[0] == "x1":
                        return ("x1", posmap[r[1]])
                    return ("x0g", r[1])

                # ---- greedy merged-final planning ----
                # run instr: in0 broadcast ref, partners at (space, lo, stride)
                finals = [i for i, po in enumerate(path_ops) if po[0] >= 2]
                d1s = [i for i, po in enumerate(path_ops) if po[0] == 1]
                unsched = set(finals)
                instrs = []  # (in0_desc, in1_desc, member_paths)
                # desc = ("bcast", ref) | ("run", space, lo, stride)
                while unsched:
                    cand_groups = {}
                    for i in unsched:
                        d, r1, r2, c, so = path_ops[i]
                        for rb, rp in ((r1, r2), (r2, r1)):
                            sp, pos = space_pos(rp)
                            cand_groups.setdefault((rb, sp), {}).setdefault(pos, i)
                    best = None
                    for (rb, sp), pmp in cand_groups.items():
                        ps = sorted(pmp)
                        for ai in range(len(ps)):
                            for bi2 in range(ai + 1, len(ps)):
                                st = ps[bi2] - ps[ai]
                                if st > CAPST[sp]:
                                    break
                                run = [ps[ai], ps[bi2]]
                                nxt = ps[bi2] + st
                                while nxt in pmp and len(run) < MAXRUN:
                                    run.append(nxt)
                                    nxt += st
                                if best is None or len(run) > best[0]:
                                    best = (
                                        len(run),
                                        (rb, sp, run[0], st, [pmp[p] for p in run]),
                                    )
                                if best[0] >= MAXRUN:
                                    break
                            if best is not None and best[0] >= MAXRUN:
                                break
                        if best is not None and best[0] >= MAXRUN:
                            break
                    if best is None or best[0] < 3:
                        break
                    _, (rb, sp, lo, st, members) = best
                    instrs.append((("bcast", rb), ("run", sp, lo, st), members))
                    unsched -= set(members)
                # pair up the leftovers (any two compatible finals share one op)
                leftover = sorted(unsched)
                usedf = [False] * len(leftover)
                for ii in range(len(leftover)):
                    if usedf[ii]:
                        continue
                    i = leftover[ii]
                    d, r1, r2, c, so = path_ops[i]
                    A = (r1, r2, i)
                    matched = False
                    for jj in range(ii + 1, len(leftover)):
                        if usedf[jj]:
                            continue
                        jp = leftover[jj]
                        d2, q1, q2, c2, so2 = path_ops[jp]
                        m = match2((r1, r2, i), (q1, q2, jp), False)
                        if m:
                            (xa, xb, pi), (ya, yb, pj) = m
                            st0 = ref_pos(ya) - ref_pos(xa)
                            st1 = ref_pos(yb) - ref_pos(xb)
                            d0 = (
                                ("bcast", xa)
                                if st0 == 0
                                else ("run", ref_space(xa), ref_pos(xa), st0)
                            )
                            d1_ = (
                                ("bcast", xb)
                                if st1 == 0
                                else ("run", ref_space(xb), ref_pos(xb), st1)
                            )
                            instrs.append((d0, d1_, [pi, pj]))
                            usedf[ii] = usedf[jj] = True
                            matched = True
                            break
                    if not matched:
                        sp, pos = space_pos(r2)
                        instrs.append((("bcast", r1), ("run", sp, pos, 1), [i]))
                        usedf[ii] = True

                def desc_ready(desc):
                    kind = desc[0]
                    if kind == "bcast":
                        r = desc[1]
                        if r[0] in ("sg", "pair"):
                            return slot_done_at.get(slot_of[r], -1)
                        return -1
                    _, sp, lo, st = desc
                    return -1  # positions handled by caller for prod

                def instr_ready(ins):
                    d0, d1_, members = ins
                    bi = -1
                    for desc, nmem in ((d0, len(members)), (d1_, len(members))):
                        if desc[0] == "bcast":
                            r = desc[1]
                            if r[0] in ("sg", "pair"):
                                bi = max(bi, slot_done_at.get(slot_of[r], -1))
                        else:
                            _, sp, lo, st = desc
                            if sp == "prod":
                                for k in range(nmem):
                                    bi = max(
                                        bi, slot_done_at.get(lo + k * st, -1)
                                    )
                    return bi

                ready_after = defaultdict(list)  # build-op idx -> events
                for i in d1s:
                    r1 = path_ops[i][1]
                    ready_after[slot_done_at.get(slot_of[r1], -1)].append(("d1", i))
                for j, ins in enumerate(instrs):
                    ready_after[instr_ready(ins)].append(("ins", j))

                # dry pass: MM emission order -> start/stop flags per so
                mm_seq = []
                for bi in range(-1, len(build_ops)):
                    for kind, j in ready_after[bi]:
                        if kind == "d1":
                            mm_seq.append(j)
                        else:
                            mm_seq.extend(instrs[j][2])
                first_for_so = {}
                last_for_so = {}
                for i in mm_seq:
                    so = path_ops[i][4]
                    if so not in first_for_so:
                        first_for_so[so] = i
                    last_for_so[so] = i
                assert len(mm_seq) == len(path_ops)

                acc = {}
                dbl = []
                with tc.tile_pool(
                    name=f"acc{sos[0]}", bufs=8, space="PSUM"
                ) as acc_pool:
                    for p in range(4):
                        t = acc_pool.tile(
                            [U, 2 * ZS],
                            mybir.dt.float32,
                            tag=f"acc{p}",
                            name=f"acc_{sos[0]}_{p}",
                            bufs=1,
                        )
                        dbl.append(t)
                    for k, so in enumerate(sos):
                        acc[so] = dbl[k // 2][:, (k % 2) * ZS : (k % 2 + 1) * ZS]

                    base_of = {
                        "prod": (prod, max_slots),
                        "x1": (x1t, S),
                        "x0g": (x0g, S),
                    }

                    def pref(r):
                        kind, key = r
                        if kind in ("x1", "x1s"):
                            return seg(x1t, posmap[key])
                        if kind == "x0g":
                            return seg(x0g, key)
                        return seg(prod, slot_of[r])

                    def ap_of(desc, n):
                        if desc[0] == "bcast":
                            a = pref(desc[1])
                            if n == 1:
                                return a
                            return a.rearrange("p (o z) -> p o z", o=1).broadcast_to(
                                [U, n, ZS]
                            )
                        _, sp, lo, st = desc
                        base, W = base_of[sp]
                        if n == 1:
                            return seg(base, lo)
                        base3 = base[:].rearrange("p (w z) -> p w z", w=W)
                        if st > 0:
                            return base3[:, lo : lo + (n - 1) * st + 1 : st, :]
                        end = lo + (n - 1) * st - 1
                        if end < 0:
                            return base3[:, lo :: st, :][:, 0:n, :]
                        return base3[:, lo : end : st, :]

                    def emit_mm(i, rhs):
                        d, r1, r2, c, so = path_ops[i]
                        gi = len(coeff_order)
                        coeff_order.append(c)
                        sj, sk = gi // SLAB, gi % SLAB
                        if slab_state["idx"] != sj:
                            slab_state["idx"] = sj
                            stt = slab_pool.tile(
                                [U, SLAB * U], dt, tag="slab", name=f"slab{sj}"
                            )
                            slab_state["tile"] = stt
                            nc.scalar.dma_start(
                                out=stt[:].rearrange("p (d c) -> p d c", d=SLAB),
                                in_=cd_d[sj * SLAB * U : (sj + 1) * SLAB * U, :]
                                .rearrange("(d p) c -> p d c", p=U),
                            )
                        stt = slab_state["tile"]
                        nc.tensor.matmul(
                            acc[so],
                            stt[:, sk * U : (sk + 1) * U],
                            rhs,
                            start=(i == first_for_so[so]),
                            stop=(i == last_for_so[so]),
                            skip_group_check=True,
                        )

                    def emit_instr(j):
                        d0, d1_, members = instrs[j]
                        n = len(members)
                        t1 = tmp_pool.tile(
                            [U, n * ZS], dt, tag=f"tmp{n}", bufs=TMP_BUFS[n],
                            name=f"t{sos[0]}_{j}",
                        )
                        out = t1[:]
                        if n > 1:
                            out = out.rearrange("p (r z) -> p r z", r=n)
                        nc.vector.tensor_tensor(
                            out=out, in0=ap_of(d0, n), in1=ap_of(d1_, n), op=MULT
                        )
                        for k, i in enumerate(members):
                            emit_mm(i, t1[:, k * ZS : (k + 1) * ZS])

                    def emit_event(ev):
                        kind, j = ev
                        if kind == "d1":
                            emit_mm(j, pref(path_ops[j][1]))
                        else:
                            emit_instr(j)

                    def emit_build(ms):
                        n = len(ms)
                        if n == 1:
                            (ra, rb_, o) = ms[0]
                            nc.vector.tensor_tensor(
                                out=seg(prod, o), in0=pref(ra), in1=pref(rb_),
                                op=MULT,
                            )
                            return
                        # derive strides from members (run or matched pair)
                        (a0, b0, o0), (a1, b1, o1) = ms[0], ms[1]
                        st_a = mref_pos(a1) - mref_pos(a0)
                        st_b = mref_pos(b1) - mref_pos(b0)
                        st_o = o1 - o0
                        da = (
                            ("bcast", a0)
                            if st_a == 0
                            else ("run", mref_space(a0), mref_pos(a0), st_a)
                        )
                        db = (
                            ("bcast", b0)
                            if st_b == 0
                            else ("run", mref_space(b0), mref_pos(b0), st_b)
                        )
                        base3 = prod[:].rearrange("p (w z) -> p w z", w=max_slots)
                        out = base3[:, o0 : o0 + (n - 1) * st_o + 1 : st_o, :]
                        nc.vector.tensor_tensor(
                            out=out, in0=ap_of(da, n), in1=ap_of(db, n), op=MULT
                        )

                    for ev in ready_after[-1]:
                        emit_event(ev)
                    for bi, ms in enumerate(build_ops):
                        emit_build(ms)
                        for ev in ready_after[bi]:
                            emit_event(ev)

                    # drain accumulators two segments at a time (one
                    # double-bank copy + one DMA per adjacent so pair),
                    # alternating ScalarE/DVE and both HWDGE queues
                    for p in range(4):
                        so = sos[2 * p]
                        ostg = tmp_pool.tile(
                            [U, 2 * ZS], dt, tag="ostg", bufs=2, name=f"ostg{so}"
                        )
                        if p % 2 == 0:
                            nc.scalar.copy(out=ostg[:], in_=dbl[p][:])
                        else:
                            nc.vector.tensor_copy(out=ostg[:], in_=dbl[p][:])
                        deng = nc.sync if p % 2 == 0 else nc.scalar
                        deng.dma_start(
                            out=out_d[so * U : (so + 2) * U, :].rearrange(
                                "(s q) z -> q s z", q=U
                            ),
                            in_=ostg[:].rearrange("q (s z) -> q s z", s=2),
                        )

    nc.compile()
    return nc, coeff_order, seg_order


def kernel(x0, x1, coeff1, coeff2, coeff3, i0, idx1, idx2, idx3):
    global LAST_EXEC_NS, LAST_RESULTS
    from concourse.bass_utils import run_bass_kernel_spmd

    x0 = np.asarray(x0, dtype=np.float32)
    x1 = np.asarray(x1, dtype=np.float32)
    i0 = np.asarray(i0).astype(np.int64)
    idxs = [np.asarray(a) for a in (idx1, idx2, idx3)]
    coeffs = [np.asarray(c, dtype=np.float32) for c in (coeff1, coeff2, coeff3)]

    dtype_name = os.environ.get("KERNEL_DTYPE", "bfloat16")
    act_frac = float(os.environ.get("KERNEL_ACT_FRAC", "0.55"))
    pool_frac = float(os.environ.get("KERNEL_POOL_FRAC", "0.3"))
    warmup = int(os.environ.get("KERNEL_WARMUP", "12"))
    gpsimd_every = int(os.environ.get("KERNEL_GPSIMD_EVERY", "0"))
    npdt = np.float32
    if dtype_name != F32:
        import ml_dtypes

        npdt = ml_dtypes.bfloat16

    groups, all_sq = _build_plan(idxs, coeffs)
    nc, coeff_order, seg_order = _build_bass(groups, dtype_name, act_frac, warmup, pool_frac, all_sq, gpsimd_every)
    n_slabs = (len(coeff_order) + SLAB - 1) // SLAB
    cdiag = np.zeros((n_slabs * SLAB * U, U), dtype=npdt)
    for gi, c in enumerate(coeff_order):
        blk = cdiag[gi * U : (gi + 1) * U, :]
        np.fill_diagonal(blk, np.asarray(c, dtype=npdt))

    in_maps = []
    eye = np.arange(NELEM)
    x0c = x0.astype(npdt)
    for c in range(NCORES):
        zl, zh = c * ZS, (c + 1) * ZS
        shard = x1[zl:zh]
        x1t = np.ascontiguousarray(
            shard.reshape(ZS, S, U).transpose(1, 2, 0).reshape(S * U, ZS)
        ).astype(npdt)
        oh = (i0[zl:zh][None, :] == eye[:, None]).astype(npdt)
        in_maps.append({"x1t": x1t, "x0w": x0c, "oh": oh, "cdiag": cdiag})

    trace = os.environ.get("BASS_TRACE", "") not in ("", "0")
    trace_cores = None
    tc_env = os.environ.get("KERNEL_TRACE_CORES", "")
    if tc_env:
        trace_cores = [int(x) for x in tc_env.split(",")]
    res = run_bass_kernel_spmd(
        nc, in_maps, core_ids=list(range(NCORES)), trace=trace,
        trace_cores=trace_cores,
    )
    LAST_EXEC_NS = res.exec_time_ns
    LAST_RESULTS = res

    out = np.empty((Z, S * U), dtype=np.float32)
    for c in range(NCORES):
        outt = np.asarray(res.results[c]["outt"], dtype=np.float32)
        out[c * ZS : (c + 1) * ZS] = (
            outt.reshape(S, U, ZS).transpose(2, 0, 1).reshape(ZS, S * U)
        )
    return out

